# revision 15
# baseline (speedup 1.0000x reference)
"""Trainium2 Bass kernel for CascadedPathEncoder.

Reference computation (per sample b):
    h_0 = relu(W_0 @ [0_256; wp_0] + b_0)
    h_p = relu(W_p @ [h_{p-1}; wp_p] + b_p)      p = 1..31
    out[b] = concat_p h_p                         -> [8192, 8192]

Strategy: pure data parallel over 8 NeuronCores (1024 batch rows each),
bf16 compute with f32 PSUM accumulation. Per core the hidden state
lives transposed in SBUF as one bf16 [128, 2, 2, 512] tile per step
(partition = hidden-within-chunk, dims = m-chunk, batch-tile, batch).

Per step each of the 4 PSUM banks (m-chunk x batch-tile) accumulates
12 full 128x128-mode matmuls: a K=128 wp matmul whose lhsT carries
zero rows for every step but p (wx packs all 32 steps' 4 wp rows into
the 128 partitions; zeros select the step), then two K=128 chunks of
the previous hidden state. Everything stays in one PE array mode:
measured, 32x128 row-tiling the wp pass is faster on paper (one
concurrent 4-tile span instead of 4 serial matmuls) but the Tile
scheduler hoists the next step's wp tiles into the middle of the
h-chain, paying 4 mode-switch drains per step -- a net loss. The
all-full-mode form is hoist-immune: the conveyor just streams.

The h matmuls run t-outer, (k0,m0),(k1,m0),(k0,m1),(k1,m1) within
each batch-tile, so banks close at slots 2/4/6/8 of the h-chain.
Bias+relu interleave: ACT takes the m0 banks (slots 2, 6), DVE the m1
banks (slots 4, 8); next-step k0 matmuls then wait only on an ACT
product and k1 only on a DVE product, hiding the relu ring under the
PE conveyor (~2.6us/step). ~120 warmup matmuls on a memset tile run
while inputs load, so the HAM clock gate (PE at 1.2 GHz until ~3.4us
of sustained activity) flips before the first real matmul. Outputs
stream per step as two 256KB DMAs from idle queues: Sync HWDGE ships
the t=0 halves, GpSimd SWDGE the t=1 halves (never the ACT engine: a
HWDGE dispatch occupies it ~0.6us). The last two steps split 3 ways
(+scalar ring) to shorten the drain tail.

Host re-assembles the full [8192, 8192] f32 from bf16 step outputs.
"""

import numpy as np
import ml_dtypes

BF16 = ml_dtypes.bfloat16

P = 32          # scan steps
PD = 4          # point dim
H = 256         # hidden dim
B = 8192        # global batch
NCORES = 8
BS = B // NCORES  # 1024 rows per core
TN = 512        # matmul moving free dim (one PSUM bank of f32; ISA max)
NT = BS // TN   # batch tiles per core
NWARM = 48      # PE warmup matmuls (HAM un-throttle before first real MM)

_CACHE = {}


def _build_nc():
    from contextlib import ExitStack

    import concourse.bass as bass
    import concourse.tile as tile
    from concourse import bacc, mybir

    dt = mybir.dt
    ts = bass.ts

    nc = bacc.Bacc(
        "TRN2", target_bir_lowering=False, debug=False, num_devices=NCORES
    )
    # wh[kk, p, k, jj] = W[p, jj + 128m, 128k + kk] (lhsT for the h chunks)
    wh = nc.dram_tensor("wh", [128, P, 2, 256], dt.bfloat16, kind="ExternalInput").ap()
    # K=128 wp lhsT with zero rows selecting the step:
    # wx[4q + r, p, m, j] = W[p, 128m + j, 256 + r] if q == p else 0
    wx = nc.dram_tensor("wx", [128, P, 2, 128], dt.bfloat16, kind="ExternalInput").ap()
    # pdx[4q + r, b] = path_data[c*BS + b, 4q + r]
    pdx = nc.dram_tensor("pdx", [128, BS], dt.bfloat16, kind="ExternalInput").ap()
    bias = nc.dram_tensor("bias", [128, P, 2], dt.float32, kind="ExternalInput").ap()
    out = nc.dram_tensor(
        "out", [P, 128, 2, NT, TN], dt.bfloat16, kind="ExternalOutput"
    ).ap()

    with tile.TileContext(nc) as tc, ExitStack() as ctx:
        const = ctx.enter_context(tc.tile_pool(name="const", bufs=1))
        state = ctx.enter_context(tc.tile_pool(name="state", bufs=10))
        psum = ctx.enter_context(tc.tile_pool(name="psum", bufs=2, space="PSUM"))

        wx_sb = const.tile([128, P, 2, 128], dt.bfloat16)
        pdx_sb = const.tile([128, BS], dt.bfloat16)
        b_sb = const.tile([128, P, 2], dt.float32)
        wh_sb = const.tile([128, P, 2, 256], dt.bfloat16)
        warm_sb = const.tile([128, 64], dt.bfloat16)

        # Input DMAs on the sync HWDGE ring, ordered by first use; all
        # transfers are full 128-partition width.
        # Transfer sizing balances two costs: each dma_start dispatch
        # occupies the sync engine ~0.6-1.2us (so few transfers), but a
        # consumer waits on its WHOLE transfer's completion semaphore
        # (so chunks small enough to not stall their first use).
        # pdx rides the idle scalar (ACT HWDGE) queue head: its semaphore
        # gates the first real matmul, and dispatching it concurrently
        # with sync's stream lands it ~2us earlier.
        nc.scalar.dma_start(out=pdx_sb[:], in_=pdx[:])
        nc.sync.dma_start(out=b_sb[:], in_=bias[:])
        nc.sync.dma_start(out=wx_sb[:, 0:2, :, :], in_=wx[:, 0:2, :, :])
        nc.sync.dma_start(out=wh_sb[:, 0:4, :, :], in_=wh[:, 0:4, :, :])
        nc.sync.dma_start(out=wx_sb[:, 2:6, :, :], in_=wx[:, 2:6, :, :])
        nc.sync.dma_start(out=wh_sb[:, 4:8, :, :], in_=wh[:, 4:8, :, :])
        nc.sync.dma_start(out=wx_sb[:, 6:16, :, :], in_=wx[:, 6:16, :, :])
        nc.sync.dma_start(out=wh_sb[:, 8:16, :, :], in_=wh[:, 8:16, :, :])
        nc.sync.dma_start(out=wx_sb[:, 16:32, :, :], in_=wx[:, 16:32, :, :])
        nc.sync.dma_start(out=wh_sb[:, 16:24, :, :], in_=wh[:, 16:24, :, :])
        nc.sync.dma_start(out=wh_sb[:, 24:32, :, :], in_=wh[:, 24:32, :, :])

        # PE warmup: flip the HAM clock gate to 8/8 while inputs stream.
        nc.vector.memset(warm_sb[:], 0.0)
        # ACT table preload: the first ACTIVATE pays a ~1.3us Relu
        # ACT_TABLE_LOAD; trigger it on scratch during the input wait.
        warm_out = const.tile([128, 8], dt.bfloat16)
        nc.scalar.activation(
            warm_out[:],
            warm_sb[:, 0:8],
            mybir.ActivationFunctionType.Relu,
            scale=1.0,
        )
        warm_ps = psum.tile([128, NT, TN], dt.float32, tag="ps_m0", name="warm")
        for i in range(NWARM):
            nc.tensor.matmul(
                warm_ps[0:64, 0, 0:64],
                lhsT=warm_sb[:],
                rhs=warm_sb[:],
                start=True,
                stop=True,
                skip_group_check=True,
            )

        h_prev = None
        for p in range(P):
            ps = [
                psum.tile(
                    [128, NT, TN],
                    dt.float32,
                    tag=f"ps_m{m}",
                    name=f"ps_p{p}m{m}",
                )
                for m in range(2)
            ]
            # wp pass: K=128 matmuls, zero lhsT rows select step p. Opens
            # each accumulation group. Full-mode: the scheduler may hoist
            # these into the previous step's stream at no cost.
            for m in range(2):
                for t in range(NT):
                    nc.tensor.matmul(
                        ps[m][:, t, :],
                        lhsT=wx_sb[:, p, m, :],
                        rhs=pdx_sb[:, ts(t, TN)],
                        start=True,
                        stop=(p == 0),
                    )
            hn = state.tile(
                [128, 2, NT, TN], dt.bfloat16, tag="h", name=f"h_p{p}"
            )

            def relu_act(t):
                nc.scalar.activation(
                    hn[:, 0, t, :],
                    ps[0][:, t, :],
                    mybir.ActivationFunctionType.Relu,
                    bias=b_sb[:, p, 0:1],
                    scale=1.0,
                )

            def relu_dve(t):
                nc.vector.tensor_scalar(
                    hn[:, 1, t, :],
                    ps[1][:, t, :],
                    scalar1=b_sb[:, p, 1:2],
                    scalar2=0.0,
                    op0=mybir.AluOpType.add,
                    op1=mybir.AluOpType.max,
                )

            if p > 0:
                # t-outer; per t: (k0,m0),(k1,m0),(k0,m1),(k1,m1) so banks
                # close at slots 2/4/6/8; relu fires as each bank closes.
                # k0 matmuls consume ACT products, k1 consume DVE products.
                # Last two steps close the t=1 banks first so their relu
                # products ship ~1.3us earlier, shortening the drain.
                t_order = range(NT) if p < P - 2 else range(NT - 1, -1, -1)
                for t in t_order:
                    for m in range(2):
                        for k in range(2):
                            nc.tensor.matmul(
                                ps[m][:, t, :],
                                lhsT=wh_sb[:, p, k, ts(m, 128)],
                                rhs=h_prev[:, k, t, :],
                                start=False,
                                stop=(k == 1),
                            )
                        if m == 0:
                            relu_act(t)
                        else:
                            relu_dve(t)
            else:
                for t in range(NT):
                    relu_act(t)
                    relu_dve(t)
            # outputs: all on HWDGE rings (sync, idle post-inputs); gpsimd
            # carries no DMAs at all, so the exit-barrier SWDGE drain
            # (measured ~3.5us when the SWDGE ring was used) vanishes
            if p >= P - 2:
                # tail: t=1 products (computed first) ship immediately,
                # each on its own ring
                nc.scalar.dma_start(out=out[p, :, 0, 1, :], in_=hn[:, 0, 1, :])
                nc.sync.dma_start(out=out[p, :, 1, 1, :], in_=hn[:, 1, 1, :])
                nc.sync.dma_start(out=out[p, :, :, 0, :], in_=hn[:, :, 0, :])
            else:
                nc.sync.dma_start(out=out[p, :, :, 0, :], in_=hn[:, :, 0, :])
                nc.sync.dma_start(out=out[p, :, :, 1, :], in_=hn[:, :, 1, :])
            h_prev = hn

    nc.compile()
    return nc


def _get_nc():
    if "nc" not in _CACHE:
        _CACHE["nc"] = _build_nc()
    return _CACHE["nc"]


def _pack_inputs(path_data, W, b):
    """Host-side packing into the DRAM layouts the kernel expects."""
    # lhsT for the two K=128 chunks: wh[kk, p, k, jj] = W[p, jj, 128k+kk]
    wh_np = np.ascontiguousarray(
        W[:, :, :H].reshape(P, H, 2, 128).transpose(3, 0, 2, 1)
    ).astype(BF16)
    # K=128 wp lhsT blocks, zero rows select the step:
    # wx[4q+r, p, m, j] = W[p, 128m+j, 256+r] if q == p else 0
    wx_np = np.zeros((128, P, 2, 128), dtype=BF16)
    wxs = W[:, :, H:].reshape(P, 2, 128, PD).transpose(3, 0, 1, 2).astype(BF16)
    for p in range(P):
        wx_np[4 * p : 4 * p + 4, p] = wxs[:, p]
    # bias[j, p, m] = b[p, 128m+j]
    b_np = np.ascontiguousarray(b.reshape(P, 2, 128).transpose(2, 0, 1)).astype(
        np.float32
    )
    # per-core rhs for the wp pass: pdx[4q+r, bb] = path_data[c*BS+bb, 4q+r]
    pdx_all = [
        np.ascontiguousarray(path_data[c * BS : (c + 1) * BS].T).astype(BF16)
        for c in range(NCORES)
    ]
    return wh_np, wx_np, b_np, pdx_all


def _make_in_maps(path_data, W, b):
    wh_np, wx_np, b_np, pdx_all = _pack_inputs(path_data, W, b)
    return [
        {"wh": wh_np, "wx": wx_np, "bias": b_np, "pdx": pdx_all[c]}
        for c in range(NCORES)
    ]


def _unpack_out(results):
    # out[p, jj, m, t, bb] -> full[c*BS + t*TN + bb, p*256 + m*128 + jj]
    return np.concatenate(
        [
            np.asarray(r["out"])
            .transpose(3, 4, 0, 2, 1)
            .reshape(BS, P * H)
            .astype(np.float32)
            for r in results
        ],
        axis=0,
    )


def kernel(path_data, W, b):
    from concourse.bass_utils import run_bass_kernel_spmd

    path_data = np.asarray(path_data, dtype=np.float32)
    W = np.asarray(W, dtype=np.float32)
    b = np.asarray(b, dtype=np.float32)

    in_maps = _make_in_maps(path_data, W, b)
    nc = _get_nc()
    res = run_bass_kernel_spmd(nc, in_maps, core_ids=list(range(NCORES)))
    return _unpack_out(res.results)


# revision 16
# speedup vs baseline: 1.0101x; 1.0101x over previous
"""Trainium2 Bass kernel for CascadedPathEncoder.

Reference computation (per sample b):
    h_0 = relu(W_0 @ [0_256; wp_0] + b_0)
    h_p = relu(W_p @ [h_{p-1}; wp_p] + b_p)      p = 1..31
    out[b] = concat_p h_p                         -> [8192, 8192]

Strategy: pure data parallel over 8 NeuronCores (1024 batch rows each),
bf16 compute with f32 PSUM accumulation. Per core the hidden state
lives transposed in SBUF as one bf16 [128, 2, 2, 512] tile per step
(partition = hidden-within-chunk, dims = m-chunk, batch-tile, batch).

Per step each of the 4 PSUM banks (m-chunk x batch-tile) accumulates
12 full 128x128-mode matmuls: a K=128 wp matmul whose lhsT carries
zero rows for every step but p (wx packs all 32 steps' 4 wp rows into
the 128 partitions; zeros select the step), then two K=128 chunks of
the previous hidden state. Everything stays in one PE array mode:
measured, 32x128 row-tiling the wp pass is faster on paper (one
concurrent 4-tile span instead of 4 serial matmuls) but the Tile
scheduler hoists the next step's wp tiles into the middle of the
h-chain, paying 4 mode-switch drains per step -- a net loss. The
all-full-mode form is hoist-immune: the conveyor just streams.

The h matmuls run t-outer, (k0,m0),(k1,m0),(k0,m1),(k1,m1) within
each batch-tile, so banks close at slots 2/4/6/8 of the h-chain.
Bias+relu interleave: ACT takes the m0 banks (slots 2, 6), DVE the m1
banks (slots 4, 8); next-step k0 matmuls then wait only on an ACT
product and k1 only on a DVE product, hiding the relu ring under the
PE conveyor (~2.6us/step). ~120 warmup matmuls on a memset tile run
while inputs load, so the HAM clock gate (PE at 1.2 GHz until ~3.4us
of sustained activity) flips before the first real matmul. Outputs
stream per step as two 256KB DMAs from idle queues: Sync HWDGE ships
the t=0 halves, GpSimd SWDGE the t=1 halves (never the ACT engine: a
HWDGE dispatch occupies it ~0.6us). The last two steps split 3 ways
(+scalar ring) to shorten the drain tail.

Host re-assembles the full [8192, 8192] f32 from bf16 step outputs.
"""

import numpy as np
import ml_dtypes

BF16 = ml_dtypes.bfloat16

P = 32          # scan steps
PD = 4          # point dim
H = 256         # hidden dim
B = 8192        # global batch
NCORES = 8
BS = B // NCORES  # 1024 rows per core
TN = 512        # matmul moving free dim (one PSUM bank of f32; ISA max)
NT = BS // TN   # batch tiles per core
NWARM = 66      # PE warmup matmuls (HAM un-throttle before first real MM)

_CACHE = {}


def _build_nc():
    from contextlib import ExitStack

    import concourse.bass as bass
    import concourse.tile as tile
    from concourse import bacc, mybir

    dt = mybir.dt
    ts = bass.ts

    nc = bacc.Bacc(
        "TRN2", target_bir_lowering=False, debug=False, num_devices=NCORES
    )
    # wh[kk, p, k, jj] = W[p, jj + 128m, 128k + kk] (lhsT for the h chunks)
    wh = nc.dram_tensor("wh", [128, P, 2, 256], dt.bfloat16, kind="ExternalInput").ap()
    # K=128 wp lhsT with zero rows selecting the step:
    # wx[4q + r, p, m, j] = W[p, 128m + j, 256 + r] if q == p else 0
    wx = nc.dram_tensor("wx", [128, P, 2, 128], dt.bfloat16, kind="ExternalInput").ap()
    # pdx[4q + r, b] = path_data[c*BS + b, 4q + r]
    pdx = nc.dram_tensor("pdx", [128, BS], dt.bfloat16, kind="ExternalInput").ap()
    bias = nc.dram_tensor("bias", [128, P, 2], dt.float32, kind="ExternalInput").ap()
    out = nc.dram_tensor(
        "out", [P, 128, 2, NT, TN], dt.bfloat16, kind="ExternalOutput"
    ).ap()

    with tile.TileContext(nc) as tc, ExitStack() as ctx:
        const = ctx.enter_context(tc.tile_pool(name="const", bufs=1))
        state = ctx.enter_context(tc.tile_pool(name="state", bufs=10))
        psum = ctx.enter_context(tc.tile_pool(name="psum", bufs=2, space="PSUM"))

        wx_sb = const.tile([128, P, 2, 128], dt.bfloat16)
        pdx_sb = const.tile([128, BS], dt.bfloat16)
        b_sb = const.tile([128, P, 2], dt.float32)
        wh_sb = const.tile([128, P, 2, 256], dt.bfloat16)
        warm_sb = const.tile([128, 64], dt.bfloat16)

        # Input DMAs on the sync HWDGE ring, ordered by first use; all
        # transfers are full 128-partition width.
        # Transfer sizing balances two costs: each dma_start dispatch
        # occupies the sync engine ~0.6-1.2us (so few transfers), but a
        # consumer waits on its WHOLE transfer's completion semaphore
        # (so chunks small enough to not stall their first use).
        # pdx rides the idle scalar (ACT HWDGE) queue head: its semaphore
        # gates the first real matmul, and dispatching it concurrently
        # with sync's stream lands it ~2us earlier.
        nc.scalar.dma_start(out=pdx_sb[:], in_=pdx[:])
        nc.sync.dma_start(out=b_sb[:], in_=bias[:])
        nc.sync.dma_start(out=wx_sb[:, 0:2, :, :], in_=wx[:, 0:2, :, :])
        nc.sync.dma_start(out=wh_sb[:, 0:4, :, :], in_=wh[:, 0:4, :, :])
        nc.sync.dma_start(out=wx_sb[:, 2:6, :, :], in_=wx[:, 2:6, :, :])
        nc.sync.dma_start(out=wh_sb[:, 4:8, :, :], in_=wh[:, 4:8, :, :])
        nc.sync.dma_start(out=wx_sb[:, 6:16, :, :], in_=wx[:, 6:16, :, :])
        nc.sync.dma_start(out=wh_sb[:, 8:16, :, :], in_=wh[:, 8:16, :, :])
        nc.sync.dma_start(out=wx_sb[:, 16:32, :, :], in_=wx[:, 16:32, :, :])
        nc.sync.dma_start(out=wh_sb[:, 16:24, :, :], in_=wh[:, 16:24, :, :])
        nc.sync.dma_start(out=wh_sb[:, 24:32, :, :], in_=wh[:, 24:32, :, :])

        # PE warmup: flip the HAM clock gate to 8/8 while inputs stream.
        nc.vector.memset(warm_sb[:], 0.0)
        # ACT table preload: the first ACTIVATE pays a ~1.3us Relu
        # ACT_TABLE_LOAD; trigger it on scratch during the input wait.
        warm_out = const.tile([128, 8], dt.bfloat16)
        nc.scalar.activation(
            warm_out[:],
            warm_sb[:, 0:8],
            mybir.ActivationFunctionType.Relu,
            scale=1.0,
        )
        warm_ps = psum.tile([128, NT, TN], dt.float32, tag="ps_m0", name="warm")
        for i in range(NWARM):
            nc.tensor.matmul(
                warm_ps[0:64, 0, 0:64],
                lhsT=warm_sb[:],
                rhs=warm_sb[:],
                start=True,
                stop=True,
                skip_group_check=True,
            )

        h_prev = None
        for p in range(P):
            ps = [
                psum.tile(
                    [128, NT, TN],
                    dt.float32,
                    tag=f"ps_m{m}",
                    name=f"ps_p{p}m{m}",
                )
                for m in range(2)
            ]
            # wp pass: K=128 matmuls, zero lhsT rows select step p. Opens
            # each accumulation group. Full-mode: the scheduler may hoist
            # these into the previous step's stream at no cost.
            for m in range(2):
                for t in range(NT):
                    nc.tensor.matmul(
                        ps[m][:, t, :],
                        lhsT=wx_sb[:, p, m, :],
                        rhs=pdx_sb[:, ts(t, TN)],
                        start=True,
                        stop=(p == 0),
                    )
            hn = state.tile(
                [128, 2, NT, TN], dt.bfloat16, tag="h", name=f"h_p{p}"
            )

            def relu_act(t):
                nc.scalar.activation(
                    hn[:, 0, t, :],
                    ps[0][:, t, :],
                    mybir.ActivationFunctionType.Relu,
                    bias=b_sb[:, p, 0:1],
                    scale=1.0,
                )

            def relu_dve(t):
                nc.vector.tensor_scalar(
                    hn[:, 1, t, :],
                    ps[1][:, t, :],
                    scalar1=b_sb[:, p, 1:2],
                    scalar2=0.0,
                    op0=mybir.AluOpType.add,
                    op1=mybir.AluOpType.max,
                )

            if p > 0:
                # t-outer; per t: (k0,m0),(k1,m0),(k0,m1),(k1,m1) so banks
                # close at slots 2/4/6/8; relu fires as each bank closes.
                # k0 matmuls consume ACT products, k1 consume DVE products.
                # Last two steps close the t=1 banks first so their relu
                # products ship ~1.3us earlier, shortening the drain.
                t_order = range(NT) if p < P - 2 else range(NT - 1, -1, -1)
                for t in t_order:
                    for m in range(2):
                        for k in range(2):
                            nc.tensor.matmul(
                                ps[m][:, t, :],
                                lhsT=wh_sb[:, p, k, ts(m, 128)],
                                rhs=h_prev[:, k, t, :],
                                start=False,
                                stop=(k == 1),
                            )
                        if m == 0:
                            relu_act(t)
                        else:
                            relu_dve(t)
            else:
                for t in range(NT):
                    relu_act(t)
                    relu_dve(t)
            # outputs: all on HWDGE rings (sync, idle post-inputs); gpsimd
            # carries no DMAs at all, so the exit-barrier SWDGE drain
            # (measured ~3.5us when the SWDGE ring was used) vanishes
            if p >= P - 2:
                # tail: t=1 products (computed first) ship immediately,
                # each on its own ring
                nc.scalar.dma_start(out=out[p, :, 0, 1, :], in_=hn[:, 0, 1, :])
                nc.sync.dma_start(out=out[p, :, 1, 1, :], in_=hn[:, 1, 1, :])
                nc.sync.dma_start(out=out[p, :, :, 0, :], in_=hn[:, :, 0, :])
            else:
                nc.sync.dma_start(out=out[p, :, :, 0, :], in_=hn[:, :, 0, :])
                nc.sync.dma_start(out=out[p, :, :, 1, :], in_=hn[:, :, 1, :])
            h_prev = hn

    nc.compile()
    return nc


def _get_nc():
    if "nc" not in _CACHE:
        _CACHE["nc"] = _build_nc()
    return _CACHE["nc"]


def _pack_inputs(path_data, W, b):
    """Host-side packing into the DRAM layouts the kernel expects."""
    # lhsT for the two K=128 chunks: wh[kk, p, k, jj] = W[p, jj, 128k+kk]
    wh_np = np.ascontiguousarray(
        W[:, :, :H].reshape(P, H, 2, 128).transpose(3, 0, 2, 1)
    ).astype(BF16)
    # K=128 wp lhsT blocks, zero rows select the step:
    # wx[4q+r, p, m, j] = W[p, 128m+j, 256+r] if q == p else 0
    wx_np = np.zeros((128, P, 2, 128), dtype=BF16)
    wxs = W[:, :, H:].reshape(P, 2, 128, PD).transpose(3, 0, 1, 2).astype(BF16)
    for p in range(P):
        wx_np[4 * p : 4 * p + 4, p] = wxs[:, p]
    # bias[j, p, m] = b[p, 128m+j]
    b_np = np.ascontiguousarray(b.reshape(P, 2, 128).transpose(2, 0, 1)).astype(
        np.float32
    )
    # per-core rhs for the wp pass: pdx[4q+r, bb] = path_data[c*BS+bb, 4q+r]
    pdx_all = [
        np.ascontiguousarray(path_data[c * BS : (c + 1) * BS].T).astype(BF16)
        for c in range(NCORES)
    ]
    return wh_np, wx_np, b_np, pdx_all


def _make_in_maps(path_data, W, b):
    wh_np, wx_np, b_np, pdx_all = _pack_inputs(path_data, W, b)
    return [
        {"wh": wh_np, "wx": wx_np, "bias": b_np, "pdx": pdx_all[c]}
        for c in range(NCORES)
    ]


def _unpack_out(results):
    # out[p, jj, m, t, bb] -> full[c*BS + t*TN + bb, p*256 + m*128 + jj]
    return np.concatenate(
        [
            np.asarray(r["out"])
            .transpose(3, 4, 0, 2, 1)
            .reshape(BS, P * H)
            .astype(np.float32)
            for r in results
        ],
        axis=0,
    )


def kernel(path_data, W, b):
    from concourse.bass_utils import run_bass_kernel_spmd

    path_data = np.asarray(path_data, dtype=np.float32)
    W = np.asarray(W, dtype=np.float32)
    b = np.asarray(b, dtype=np.float32)

    in_maps = _make_in_maps(path_data, W, b)
    nc = _get_nc()
    res = run_bass_kernel_spmd(nc, in_maps, core_ids=list(range(NCORES)))
    return _unpack_out(res.results)
